# revision 1
# baseline (speedup 1.0000x reference)
"""DistMult edge scoring on TRN2 via dma_gather bank-bucketing (fast path).

Edges bucketed by (src_bank, dst_bank), banks of 32768 rows so dma_gather's
int16 in-bank indices work. Each bucket is padded to a multiple of 128 (pad
idx 0 -> harmless garbage rows, scores dropped on host) and gathered with two
dma_gather calls (u, v) on round-robin SWDGE queues. Per-edge W rows are
host-materialized in the gather-interleaved layout and streamed per bucket.
Output layout is dma_gather's partition-interleave; host undoes it.
"""

import sys

sys.path.insert(0, "/opt/trn_rl_repo")

import numpy as np

N_NODES = 500000
N_HID = 64
N_RELS = 10
N_CORES = 8
P = 128
BANK = 32768
NBANK = (N_NODES + BANK - 1) // BANK  # 16
NB = NBANK * NBANK  # 256 buckets


def _build_program2(L, L16, n_bufs=6):
    """L: [NB] per-bucket padded edge counts (multiples of 128, shared by all
    cores)."""
    from contextlib import ExitStack

    from concourse import bass, bacc, mybir

    f32 = mybir.dt.float32
    i16 = mybir.dt.int16

    L = [int(x) for x in L]
    L16 = [int(x) for x in L16]
    Etot = sum(L)
    CT = Etot // P  # total columns
    col_base = np.concatenate([[0], np.cumsum(L) // P]).astype(int)
    Cmax = max(max(x // P for x in L), 1)
    IDXT = Etot // 16  # idx elements per partition, total

    nc = bacc.Bacc("TRN2", num_swdge_queues=4)
    h = nc.declare_dram_parameter("h", [N_NODES, N_HID], f32, isOutput=False)
    usrc = nc.declare_dram_parameter("usrc", [P, IDXT], i16, isOutput=False)
    vdst = nc.declare_dram_parameter("vdst", [P, IDXT], i16, isOutput=False)
    wt = nc.declare_dram_parameter("wt", [P, CT * N_HID], f32, isOutput=False)
    out = nc.declare_dram_parameter("out", [P, CT], f32, isOutput=True)

    B = n_bufs
    with ExitStack() as es:
        pre = es.enter_context(nc.semaphore("pre"))
        dma_sems = [es.enter_context(nc.semaphore(f"dma{i}")) for i in range(B)]
        ws_sems = [es.enter_context(nc.semaphore(f"ws{i}")) for i in range(B)]
        dve_sem = es.enter_context(nc.semaphore("dve_sem"))
        act_sem = es.enter_context(nc.semaphore("act_sem"))
        st_sem = es.enter_context(nc.semaphore("st_sem"))
        usrc_sb = es.enter_context(nc.sbuf_tensor("usrc_sb", [P, IDXT], i16))
        vdst_sb = es.enter_context(nc.sbuf_tensor("vdst_sb", [P, IDXT], i16))
        scores = es.enter_context(nc.sbuf_tensor("scores", [P, CT], f32))
        u_sb = [
            es.enter_context(nc.sbuf_tensor(f"u{i}", [P, Cmax * N_HID], f32))
            for i in range(B)
        ]
        v_sb = [
            es.enter_context(nc.sbuf_tensor(f"v{i}", [P, Cmax * N_HID], f32))
            for i in range(B)
        ]
        w_sb = [
            es.enter_context(nc.sbuf_tensor(f"w{i}", [P, Cmax * N_HID], f32))
            for i in range(B)
        ]

        buckets = [b for b in range(NB) if L[b] > 0]

        with nc.Block() as block:

            @block.sync
            def _(sync):
                sync.dma_start(out=usrc_sb[:], in_=usrc[:]).then_inc(pre, 16)
                sync.dma_start(out=vdst_sb[:], in_=vdst[:]).then_inc(pre, 16)
                for i, b in enumerate(buckets):
                    s = i % B
                    C = L[b] // P
                    if i >= B:
                        sync.wait_ge(dve_sem, 3 * (i - B + 1))
                    sync.dma_start(
                        out=w_sb[s][:, : C * N_HID],
                        in_=wt[:, col_base[b] * N_HID : (col_base[b] + C) * N_HID],
                    ).then_inc(ws_sems[s], 16)
                sync.wait_ge(act_sem, 1)
                sync.dma_start(out=out[:], in_=scores[:]).then_inc(st_sem, 16)
                sync.wait_ge(st_sem, 16)

            @block.gpsimd
            def _(pool):
                pool.wait_ge(pre, 32)
                for i, b in enumerate(buckets):
                    s = i % B
                    C = L[b] // P
                    sbank, dbank = b // NBANK, b % NBANK
                    sbase = sbank * BANK
                    dbase = dbank * BANK
                    if i >= B:
                        pool.wait_ge(dve_sem, 3 * (i - B + 1))
                    i16ofs = (col_base[b] * P) // 16
                    pool.dma_gather(
                        out_ap=u_sb[s][:, : C * N_HID].rearrange(
                            "p (c d) -> p c d", d=N_HID
                        ),
                        in_ap=h[sbase : min(sbase + BANK, N_NODES), :],
                        idxs_ap=usrc_sb[:, i16ofs : i16ofs + L16[b] // 16],
                        num_idxs=L16[b],
                        num_idxs_reg=L16[b],
                        elem_size=N_HID,
                        single_packet=False,
                        queue_num=(2 * i) % 4,
                    ).then_inc(dma_sems[s], 16)
                    pool.dma_gather(
                        out_ap=v_sb[s][:, : C * N_HID].rearrange(
                            "p (c d) -> p c d", d=N_HID
                        ),
                        in_ap=h[dbase : min(dbase + BANK, N_NODES), :],
                        idxs_ap=vdst_sb[:, i16ofs : i16ofs + L16[b] // 16],
                        num_idxs=L16[b],
                        num_idxs_reg=L16[b],
                        elem_size=N_HID,
                        single_packet=False,
                        queue_num=(2 * i + 1) % 4,
                    ).then_inc(dma_sems[s], 16)

            @block.vector
            def _(dve):
                nd = 0
                for i, b in enumerate(buckets):
                    s = i % B
                    C = L[b] // P
                    dve.wait_ge(dma_sems[s], 32 * (i // B + 1))
                    dve.wait_ge(ws_sems[s], 16 * (i // B + 1))
                    u3 = u_sb[s][:, : C * N_HID].rearrange("p (c d) -> p c d", d=N_HID)
                    dve.tensor_tensor(
                        out=u_sb[s][:, : C * N_HID],
                        in0=u_sb[s][:, : C * N_HID],
                        in1=v_sb[s][:, : C * N_HID],
                        op=mybir.AluOpType.mult,
                    ).then_inc(dve_sem, 1)
                    nd += 1
                    dve.wait_ge(dve_sem, nd)
                    dve.tensor_tensor(
                        out=u3,
                        in0=u3,
                        in1=w_sb[s][:, : C * N_HID].rearrange(
                            "p (c d) -> p c d", d=N_HID
                        ),
                        op=mybir.AluOpType.mult,
                    ).then_inc(dve_sem, 1)
                    nd += 1
                    dve.wait_ge(dve_sem, nd)
                    dve.tensor_reduce(
                        out=scores[:, col_base[b] : col_base[b] + C],
                        in_=u3,
                        axis=mybir.AxisListType.X,
                        op=mybir.AluOpType.add,
                    ).then_inc(dve_sem, 1)
                    nd += 1

            @block.scalar
            def _(act):
                act.wait_ge(dve_sem, 3 * len(buckets))
                act.activation(
                    out=scores[:],
                    in_=scores[:],
                    func=mybir.ActivationFunctionType.Sigmoid,
                ).then_inc(act_sem, 1)

    nc.compile()
    return nc


from concourse import mybir  # noqa: E402  (after sys.path insert)


def _wrap16(vals):
    """[n] -> [16, n/16] wrapped (idx j at (j%16, j//16)), replicated to 128."""
    n = vals.shape[0]
    w = vals.reshape(n // 16, 16).T
    return np.tile(w, (8, 1))


def kernel(h, W, src_idx, dst_idx, rel_idx):
    from concourse.bass_utils import run_bass_kernel_spmd

    h = np.ascontiguousarray(np.asarray(h, dtype=np.float32))
    W = np.ascontiguousarray(np.asarray(W, dtype=np.float32))
    src = np.asarray(src_idx).astype(np.int64)
    dst = np.asarray(dst_idx).astype(np.int64)
    rel = np.asarray(rel_idx).astype(np.int64)

    E = src.shape[0]
    esh = E // N_CORES

    orders, counts_all = [], []
    for i in range(N_CORES):
        sl = slice(i * esh, (i + 1) * esh)
        key = (src[sl] >> 15) * NBANK + (dst[sl] >> 15)
        order = np.argsort(key, kind="stable")
        counts = np.bincount(key, minlength=NB)
        orders.append(order)
        counts_all.append(counts)

    Lmax = np.maximum.reduce(counts_all)
    L = ((Lmax + P - 1) // P) * P  # per-bucket padded length, shared
    # descriptors actually issued per bucket: only 16-aligned, not 128
    L16 = np.where(Lmax > 0, ((Lmax + 15) // 16) * 16, 0)
    Etot = int(L.sum())
    CT = Etot // P
    col_base = np.concatenate([[0], np.cumsum(L) // P]).astype(int)

    in_maps, metas = [], []
    for i in range(N_CORES):
        sl = slice(i * esh, (i + 1) * esh)
        s_sh, d_sh, r_sh = src[sl], dst[sl], rel[sl]
        order, counts = orders[i], counts_all[i]
        s_loc = np.zeros(Etot, dtype=np.int16)
        d_loc = np.zeros(Etot, dtype=np.int16)
        r_pad = np.zeros(Etot, dtype=np.int64)
        origpos = np.full(Etot, -1, dtype=np.int64)
        off = 0
        for b in range(NB):
            c = int(counts[b])
            base = col_base[b] * P
            if c:
                idxs = order[off : off + c]
                off += c
                s_loc[base : base + c] = (s_sh[idxs] - (b // NBANK) * BANK).astype(
                    np.int16
                )
                d_loc[base : base + c] = (d_sh[idxs] - (b % NBANK) * BANK).astype(
                    np.int16
                )
                r_pad[base : base + c] = r_sh[idxs]
                origpos[base : base + c] = idxs
        # interleaved layouts
        usrc = _wrap16(s_loc)
        vdst = _wrap16(d_loc)
        # rel grid in (p, col) layout: slot j -> (j%128, j//128)
        rel_grid = r_pad.reshape(CT, P).T  # [128, CT]
        wt = np.ascontiguousarray(W[rel_grid].reshape(P, CT * N_HID))
        in_maps.append({"h": h, "usrc": usrc, "vdst": vdst, "wt": wt})
        metas.append(origpos)

    key = tuple(int(x) for x in L) + tuple(int(x) for x in L16)
    if key not in _PROGRAM_CACHE:
        _PROGRAM_CACHE[key] = _build_program2(L, L16)
    nc = _PROGRAM_CACHE[key]

    res = run_bass_kernel_spmd(
        nc, in_maps, core_ids=list(range(N_CORES)), trace=TRACE
    )
    global LAST_RESULT
    LAST_RESULT = res

    out_full = np.empty(E, dtype=np.float32)
    for i in range(N_CORES):
        arr = np.asarray(res.results[i]["out"])  # [128, CT]
        s_lin = arr.T.reshape(-1)  # slot j = (j%128, j//128) -> arr[p, c]
        origpos = metas[i]
        m = origpos >= 0
        out_full[i * esh + origpos[m]] = s_lin[m]
    return out_full


_PROGRAM_CACHE = {}
TRACE = False
LAST_RESULT = None



# revision 3
# speedup vs baseline: 2.9116x; 2.9116x over previous
"""DistMult edge scoring on TRN2 via dense rel-sorted pair streaming.

Host does layout only (no arithmetic on values): casts h/W to bf16, sorts each
core's edge list by relation, and materializes a dense per-edge operand stream
pairs[slot] = (h[src] | h[dst]) in the partition-interleaved layout (slot j ->
partition j%128, column j//128, 128 bf16 values per column block). Relation
runs are padded to whole 128-slot columns (shared across cores), so W[r] can
be applied on-device as a column-broadcast multiply.

Device per core: stream pair tiles (dense DMA, no gather descriptors), then on
DVE: p = u*v and p *= W[r] via scalar_tensor_tensor (4x bf16 perf mode), then
a log2 halving-add tree reduces the 64 products to a score per edge; ACT
applies sigmoid; scores stored as [128, CT] f32. Host unpermutes.
"""

import sys

sys.path.insert(0, "/opt/trn_rl_repo")

import numpy as np
import ml_dtypes

N_NODES = 500000
N_HID = 64
N_RELS = 10
N_CORES = 8
P = 128
TC = 128  # columns (of 128 slots) per tile


def _build_program(L, n_bufs=4):
    """L: [N_RELS] per-relation padded slot counts (multiples of 128, shared
    by all cores)."""
    from contextlib import ExitStack

    from concourse import bass, bacc, mybir

    f32 = mybir.dt.float32
    bf16 = mybir.dt.bfloat16

    L = [int(x) for x in L]
    CT = sum(L) // P  # total columns
    rel_col = np.concatenate([[0], np.cumsum(L) // P]).astype(int)
    T = (CT + TC - 1) // TC  # tiles
    B = n_bufs

    # per-tile list of (c0, c1, r) relation segments, columns relative to tile
    tiles = []
    for t in range(T):
        t0, t1 = t * TC, min((t + 1) * TC, CT)
        segs = []
        for r in range(N_RELS):
            a, b = max(t0, rel_col[r]), min(t1, rel_col[r + 1])
            if a < b:
                segs.append((a - t0, b - t0, r))
        tiles.append((t0, t1, segs))

    nc = bacc.Bacc("TRN2")
    ps = nc.declare_dram_parameter("ps", [P, CT * P], bf16, isOutput=False)
    wr = nc.declare_dram_parameter("wr", [P, N_RELS * N_HID], bf16, isOutput=False)
    out = nc.declare_dram_parameter("out", [P, CT], f32, isOutput=True)

    with ExitStack() as es:
        pre = es.enter_context(nc.semaphore("pre"))
        dma_sems = [es.enter_context(nc.semaphore(f"dma{i}")) for i in range(B)]
        dve_sem = es.enter_context(nc.semaphore("dve_sem"))
        act_sem = es.enter_context(nc.semaphore("act_sem"))
        st_sem = es.enter_context(nc.semaphore("st_sem"))
        w_sb = es.enter_context(nc.sbuf_tensor("w_sb", [P, N_RELS * N_HID], bf16))
        sc_bf = es.enter_context(nc.sbuf_tensor("sc_bf", [P, CT], bf16))
        sc_f = es.enter_context(nc.sbuf_tensor("sc_f", [P, CT], f32))
        bufs = [
            es.enter_context(nc.sbuf_tensor(f"tile{i}", [P, TC * P], bf16))
            for i in range(B)
        ]

        with nc.Block() as block:

            @block.sync
            def _(sync):
                sync.dma_start(out=w_sb[:], in_=wr[:]).then_inc(pre, 16)
                for t, (t0, t1, _segs) in enumerate(tiles):
                    if t >= B:
                        sync.wait_ge(dve_sem, t - B + 1)
                    cols = t1 - t0
                    sync.dma_start(
                        out=bufs[t % B][:, : cols * P],
                        in_=ps[:, t0 * P : t1 * P],
                    ).then_inc(dma_sems[t % B], 16)
                sync.wait_ge(act_sem, 1)
                sync.dma_start(out=out[:], in_=sc_f[:]).then_inc(st_sem, 16)
                sync.wait_ge(st_sem, 16)

            @block.vector
            def _(dve):
                dve.wait_ge(pre, 16)
                mult = mybir.AluOpType.mult
                add = mybir.AluOpType.add
                for t, (t0, t1, segs) in enumerate(tiles):
                    dve.wait_ge(dma_sems[t % B], 16 * (t // B + 1))
                    cols = t1 - t0
                    t3 = bufs[t % B][:, : cols * P].rearrange(
                        "p (c x) -> p c x", x=P
                    )
                    u = t3[:, :, 0:N_HID]
                    v = t3[:, :, N_HID : 2 * N_HID]
                    # p = u * v  (in-place into u's half)
                    dve.scalar_tensor_tensor(
                        out=u, in0=u, scalar=1.0, in1=v, op0=mult, op1=mult
                    )
                    # p *= W[r] broadcast over each relation's column range
                    for c0, c1, r in segs:
                        wb = (
                            w_sb[:, r * N_HID : (r + 1) * N_HID]
                            .unsqueeze(1)
                            .broadcast_to([P, c1 - c0, N_HID])
                        )
                        seg = t3[:, c0:c1, 0:N_HID]
                        dve.scalar_tensor_tensor(
                            out=seg, in0=seg, scalar=1.0, in1=wb, op0=mult, op1=mult
                        )
                    # halving-add tree over the 64 products
                    w2 = N_HID // 2
                    while w2 >= 2:
                        dve.scalar_tensor_tensor(
                            out=t3[:, :, 0:w2],
                            in0=t3[:, :, 0:w2],
                            scalar=1.0,
                            in1=t3[:, :, w2 : 2 * w2],
                            op0=mult,
                            op1=add,
                        )
                        w2 //= 2
                    dve.scalar_tensor_tensor(
                        out=sc_bf[:, t0:t1].rearrange("p (c x) -> p c x", x=1),
                        in0=t3[:, :, 0:1],
                        scalar=1.0,
                        in1=t3[:, :, 1:2],
                        op0=mult,
                        op1=add,
                    ).then_inc(dve_sem, 1)

            @block.scalar
            def _(act):
                act.wait_ge(dve_sem, T)
                act.activation(
                    out=sc_f[:],
                    in_=sc_bf[:],
                    func=mybir.ActivationFunctionType.Sigmoid,
                ).then_inc(act_sem, 1)

    nc.compile()
    return nc


def kernel(h, W, src_idx, dst_idx, rel_idx):
    from concourse.bass_utils import run_bass_kernel_spmd

    bf16 = ml_dtypes.bfloat16
    h_bf = np.asarray(h, dtype=np.float32).astype(bf16)
    W_bf = np.asarray(W, dtype=np.float32).astype(bf16)
    src = np.asarray(src_idx).astype(np.int64)
    dst = np.asarray(dst_idx).astype(np.int64)
    rel = np.asarray(rel_idx).astype(np.int64)

    E = src.shape[0]
    esh = E // N_CORES

    orders, counts_all = [], []
    for i in range(N_CORES):
        sl = slice(i * esh, (i + 1) * esh)
        order = np.argsort(rel[sl], kind="stable")
        counts = np.bincount(rel[sl], minlength=N_RELS)
        orders.append(order)
        counts_all.append(counts)

    Lmax = np.maximum.reduce(counts_all)
    L = ((Lmax + P - 1) // P) * P  # per-rel padded slots, shared by all cores
    Etot = int(L.sum())
    CT = Etot // P
    rel_base = np.concatenate([[0], np.cumsum(L)]).astype(int)

    wr = np.ascontiguousarray(
        np.broadcast_to(W_bf.reshape(1, N_RELS * N_HID), (P, N_RELS * N_HID))
    )

    in_maps, metas = [], []
    for i in range(N_CORES):
        sl = slice(i * esh, (i + 1) * esh)
        order, counts = orders[i], counts_all[i]
        s_srt = src[sl][order]
        d_srt = dst[sl][order]
        # slot of k-th sorted edge: rel_base[r] + within-rel rank
        starts = np.concatenate([[0], np.cumsum(counts[:-1])])
        ranks = np.arange(esh) - np.repeat(starts, counts)
        slots = np.repeat(rel_base[:-1], counts) + ranks
        rows = np.zeros((Etot, 2 * N_HID), dtype=bf16)
        rows[slots, :N_HID] = h_bf[s_srt]
        rows[slots, N_HID:] = h_bf[d_srt]
        ps = np.ascontiguousarray(
            rows.reshape(CT, P, 2 * N_HID).transpose(1, 0, 2).reshape(P, CT * 2 * N_HID)
        )
        in_maps.append({"ps": ps, "wr": wr})
        metas.append((order, slots))

    key = tuple(int(x) for x in L)
    if key not in _PROGRAM_CACHE:
        _PROGRAM_CACHE[key] = _build_program(L)
    nc = _PROGRAM_CACHE[key]

    res = run_bass_kernel_spmd(
        nc, in_maps, core_ids=list(range(N_CORES)), trace=TRACE
    )
    global LAST_RESULT
    LAST_RESULT = res

    out_full = np.empty(E, dtype=np.float32)
    for i in range(N_CORES):
        arr = np.asarray(res.results[i]["out"])  # [128, CT]
        s_lin = arr.T.reshape(-1)  # slot j = (j%128, j//128) -> arr[p, c]
        order, slots = metas[i]
        out_full[i * esh + order] = s_lin[slots]
    return out_full


_PROGRAM_CACHE = {}
TRACE = False
LAST_RESULT = None


# revision 4
# speedup vs baseline: 5.0143x; 1.7222x over previous
"""DistMult edge scoring on TRN2 via transposed pair streaming + PE reduce.

Host does layout only (no arithmetic on values): casts h/W to bf16, sorts each
core's edges by relation, and materializes two dense operand planes in a
feature-on-partition pair layout: column c holds edges 2c and 2c+1; partition
p = 64*(edge parity) + feature. uplane carries h[src] rows, vplane h[dst].
Relation runs are padded to whole 128-slot (64-column) boundaries, shared
across cores.

Device per core: stream plane tiles (dense DMA, no gather descriptors). DVE
does ONE fused pass q = (u * w_ptr) * v via scalar_tensor_tensor, where w_ptr
is a per-partition scalar W[r, p%64] selected per relation run. PE reduces the
64 features per edge with a matmul against a fixed [128,2] halves-summing
stationary into PSUM [2, 512] chunks. ACT evacuates PSUM with fused Sigmoid
into f32, and gpsimd issues the output stores. Host unpermutes.
"""

import sys

sys.path.insert(0, "/opt/trn_rl_repo")

import numpy as np
import ml_dtypes

N_NODES = 500000
N_HID = 64
N_RELS = 10
N_CORES = 8
P = 128
TCC = 4096  # columns (edge pairs) per DMA tile
MM = 512  # columns per matmul chunk (PSUM bank)
EV = 2048  # columns per ACT evacuation (4 banks)


def _build_program(L, n_bufs=4):
    """L: [N_RELS] per-relation padded slot counts (each a multiple of 128,
    summing to a multiple of 2*TCC, shared by all cores)."""
    from contextlib import ExitStack

    from concourse import bass, bacc, mybir

    f32 = mybir.dt.float32
    bf16 = mybir.dt.bfloat16

    L = [int(x) for x in L]
    Etot = sum(L)
    SL = Etot // 2  # columns (edge pairs)
    assert SL % TCC == 0
    T = SL // TCC  # DMA tiles
    NMM = SL // MM  # matmul chunks
    NEV = SL // EV  # evacuation groups
    B = n_bufs
    rel_col = np.concatenate([[0], np.cumsum(L) // 2]).astype(int)  # col bounds

    # per-tile list of (c0, c1, r) relation segments, columns relative to tile
    tiles = []
    for t in range(T):
        t0, t1 = t * TCC, (t + 1) * TCC
        segs = []
        for r in range(N_RELS):
            a, b = max(t0, rel_col[r]), min(t1, rel_col[r + 1])
            if a < b:
                segs.append((a - t0, b - t0, r))
        tiles.append(segs)

    nc = bacc.Bacc("TRN2")
    ups = nc.declare_dram_parameter("ups", [P, SL], bf16, isOutput=False)
    vps = nc.declare_dram_parameter("vps", [P, SL], bf16, isOutput=False)
    wcol = nc.declare_dram_parameter("wcol", [P, N_RELS], f32, isOutput=False)
    lhs = nc.declare_dram_parameter("lhs", [P, 2], bf16, isOutput=False)
    out = nc.declare_dram_parameter("out", [2, SL], f32, isOutput=True)

    with ExitStack() as es:
        pre = es.enter_context(nc.semaphore("pre"))
        dma_sems = [es.enter_context(nc.semaphore(f"dma{i}")) for i in range(B)]
        dve_sem = es.enter_context(nc.semaphore("dve_sem"))
        pe_sem = es.enter_context(nc.semaphore("pe_sem"))
        act_sem = es.enter_context(nc.semaphore("act_sem"))
        st_sem = es.enter_context(nc.semaphore("st_sem"))
        w_sb = es.enter_context(nc.sbuf_tensor("w_sb", [P, N_RELS], f32))
        lhs_sb = es.enter_context(nc.sbuf_tensor("lhs_sb", [P, 2], bf16))
        ev_sb = [
            es.enter_context(nc.sbuf_tensor(f"ev{i}", [2, EV], f32)) for i in range(2)
        ]
        u_sb = [
            es.enter_context(nc.sbuf_tensor(f"u{i}", [P, TCC], bf16)) for i in range(B)
        ]
        v_sb = [
            es.enter_context(nc.sbuf_tensor(f"v{i}", [P, TCC], bf16)) for i in range(B)
        ]
        psum = es.enter_context(nc.psum_tensor("ps", [P, 2 * EV], f32))

        with nc.Block() as block:

            @block.sync
            def _(sync):
                sync.dma_start(out=w_sb[:], in_=wcol[:]).then_inc(pre, 16)
                sync.dma_start(out=lhs_sb[:], in_=lhs[:]).then_inc(pre, 16)
                for t in range(T):
                    if t >= B:
                        # tile t-B fully consumed once its 8 matmul chunks ran
                        sync.wait_ge(pe_sem, (t - B + 1) * (TCC // MM))
                    sync.dma_start(
                        out=u_sb[t % B][:], in_=ups[:, t * TCC : (t + 1) * TCC]
                    ).then_inc(dma_sems[t % B], 16)
                    sync.dma_start(
                        out=v_sb[t % B][:], in_=vps[:, t * TCC : (t + 1) * TCC]
                    ).then_inc(dma_sems[t % B], 16)

            @block.vector
            def _(dve):
                dve.wait_ge(pre, 32)
                mult = mybir.AluOpType.mult
                for t, segs in enumerate(tiles):
                    dve.wait_ge(dma_sems[t % B], 32 * (t // B + 1))
                    last = None
                    for c0, c1, r in segs:
                        last = dve.scalar_tensor_tensor(
                            out=u_sb[t % B][:, c0:c1],
                            in0=u_sb[t % B][:, c0:c1],
                            scalar=w_sb[:, r : r + 1],
                            in1=v_sb[t % B][:, c0:c1],
                            op0=mult,
                            op1=mult,
                        )
                    last.then_inc(dve_sem, 1)

            @block.tensor
            def _(pe):
                for m in range(NMM):
                    t = m * MM // TCC
                    g = m // (EV // MM)  # evacuation group
                    pe.wait_ge(dve_sem, t + 1)
                    if g >= 2:
                        pe.wait_ge(act_sem, g - 1)
                    c0 = m * MM
                    pe.matmul(
                        psum[0:2, (c0 % (2 * EV)) : (c0 % (2 * EV)) + MM],
                        lhs_sb[:],
                        u_sb[(c0 // TCC) % B][:, (c0 % TCC) : (c0 % TCC) + MM],
                    ).then_inc(pe_sem, 1)

            @block.scalar
            def _(act):
                for e in range(NEV):
                    act.wait_ge(pe_sem, (e + 1) * (EV // MM))
                    if e >= 2:
                        act.wait_ge(st_sem, 16 * (e - 1))
                    p0 = (e % 2) * EV
                    act.activation(
                        out=ev_sb[e % 2][:],
                        in_=psum[0:2, p0 : p0 + EV],
                        func=mybir.ActivationFunctionType.Sigmoid,
                    ).then_inc(act_sem, 1)

            @block.gpsimd
            def _(gp):
                for e in range(NEV):
                    gp.wait_ge(act_sem, e + 1)
                    gp.dma_start(
                        out=out[:, e * EV : (e + 1) * EV], in_=ev_sb[e % 2][:]
                    ).then_inc(st_sem, 16)
                gp.wait_ge(st_sem, 16 * NEV)

    nc.compile()
    return nc


def kernel(h, W, src_idx, dst_idx, rel_idx):
    from concourse.bass_utils import run_bass_kernel_spmd

    bf16 = ml_dtypes.bfloat16
    h_bf = np.asarray(h, dtype=np.float32).astype(bf16)
    W_f = np.asarray(W, dtype=np.float32)
    src = np.asarray(src_idx).astype(np.int64)
    dst = np.asarray(dst_idx).astype(np.int64)
    rel = np.asarray(rel_idx).astype(np.int64)

    E = src.shape[0]
    esh = E // N_CORES

    orders, counts_all = [], []
    for i in range(N_CORES):
        sl = slice(i * esh, (i + 1) * esh)
        orders.append(np.argsort(rel[sl], kind="stable"))
        counts_all.append(np.bincount(rel[sl], minlength=N_RELS))

    Lmax = np.maximum.reduce(counts_all)
    L = ((Lmax + P - 1) // P) * P  # per-rel padded slots, shared by all cores
    # pad the last relation so total slots are a multiple of 2*TCC
    Etot = int(L.sum())
    padded = ((Etot + 2 * TCC - 1) // (2 * TCC)) * (2 * TCC)
    L[-1] += padded - Etot
    Etot = padded
    SL = Etot // 2
    rel_base = np.concatenate([[0], np.cumsum(L)]).astype(int)

    # per-partition W scalar: wcol[64*par + d, r] = W[r, d]
    wcol = np.ascontiguousarray(np.tile(W_f.T, (2, 1)))  # [128, 10] f32
    # halves-summing stationary: lhs[k, m] = 1 if k//64 == m
    lhs = np.zeros((P, 2), dtype=bf16)
    lhs[:N_HID, 0] = 1
    lhs[N_HID:, 1] = 1

    in_maps, metas = [], []
    for i in range(N_CORES):
        sl = slice(i * esh, (i + 1) * esh)
        order, counts = orders[i], counts_all[i]
        s_srt = src[sl][order]
        d_srt = dst[sl][order]
        # slot of k-th sorted edge: rel_base[r] + within-rel rank
        starts = np.concatenate([[0], np.cumsum(counts[:-1])])
        ranks = np.arange(esh) - np.repeat(starts, counts)
        slots = np.repeat(rel_base[:-1], counts) + ranks
        rows_u = np.zeros((Etot, N_HID), dtype=bf16)
        rows_v = np.zeros((Etot, N_HID), dtype=bf16)
        rows_u[slots] = h_bf[s_srt]
        rows_v[slots] = h_bf[d_srt]
        # [Etot, 64] -> [SL, 2, 64] -> [2*64, SL]
        ups = np.ascontiguousarray(
            rows_u.reshape(SL, 2, N_HID).transpose(1, 2, 0).reshape(P, SL)
        )
        vps = np.ascontiguousarray(
            rows_v.reshape(SL, 2, N_HID).transpose(1, 2, 0).reshape(P, SL)
        )
        in_maps.append({"ups": ups, "vps": vps, "wcol": wcol, "lhs": lhs})
        metas.append((order, slots))

    key = tuple(int(x) for x in L)
    if key not in _PROGRAM_CACHE:
        _PROGRAM_CACHE[key] = _build_program(L)
    nc = _PROGRAM_CACHE[key]

    res = run_bass_kernel_spmd(
        nc, in_maps, core_ids=list(range(N_CORES)), trace=TRACE
    )
    global LAST_RESULT
    LAST_RESULT = res

    out_full = np.empty(E, dtype=np.float32)
    for i in range(N_CORES):
        arr = np.asarray(res.results[i]["out"])  # [2, SL]
        s_lin = arr.T.reshape(-1)  # slot j = (j%2, j//2) -> arr[par, c]
        order, slots = metas[i]
        out_full[i * esh + order] = s_lin[slots]
    return out_full


_PROGRAM_CACHE = {}
TRACE = False
LAST_RESULT = None
